# revision 20
# baseline (speedup 1.0000x reference)
"""Trainium2 Bass kernel: MultiHeadAttention + residual + LayerNorm.

Problem shapes (hardcoded):
  x: (2, 2048, 1024) f32, 16 heads x 64 head_dim, scale = 64**-0.5
  y = LayerNorm(x + MHA(x))

Sharding: token-parallel over 8 cores. Core c handles batch b=c//4 and
query tokens [512*(c%4), 512*(c%4+1)) of that batch. Each core receives
its batch's full token sequence ROTATED so that its own 512 query tokens
are rows 0..511 (attention is permutation-invariant over keys, so K/V
token order does not matter). No cross-core collectives needed.

Datapath: fp8e4 (e4m3) matmul operands everywhere, DoubleRow perf mode
(2 contraction k-tiles per instruction; weights AP [K,2,M] must have
subtile step %16==0) for the 1024-contraction projections and the
256-key AV matmuls. Scores are single-rate fp8 (64-dim contraction
cannot DoubleRow). f32 PSUM accumulation; softmax exp on ScalarE from
PSUM with the 1/8 scale folded in; residual + LayerNorm in f32.

Schedule: next pair's K/V projection chunks are interleaved into the
current pair's attention groups so the PE stream never drains while
ScalarE works through the exps; AV lags scores by one group so the
normalize of pair p overlaps the first scores of pair p+1.

Scale management (fp8 dynamic range):
  W{q,k,v,o} stored x32 (entries ~U(+-1/32) -> ~U(+-1))
  q,k casts multiply PSUM by 1/32 (+bias)
  v cast multiplies by 1/16 (+2*bias) -> stored V = 2*v_true
  va "ones" column = 1/32 -> pav[64] = denom/32; fast recip -> 32/denom
  outT = (2*numer)*(32/denom) = 64*attn_out (sigma ~0.9, fp8-friendly)
  out-proj PSUM is 64*32 = 2048x true value; final cast multiplies 1/2048
  bo is folded into the residual input host-side (xq = x + bo)
"""

import sys

sys.path.insert(0, "/opt/trn_rl_repo")

import numpy as np
import ml_dtypes

import concourse.bass as bass
import concourse.bacc as bacc
import concourse.mybir as mybir
import concourse.tile as tile
from concourse import bass_utils
from concourse.masks import make_identity

# ---- problem constants ----
B = 2
S = 2048
D = 1024
H = 16
DH = 64
SCALE = DH ** -0.5
EPS = 1e-5

N_CORES = 8
CORES_PER_BATCH = N_CORES // B
TQ = S // CORES_PER_BATCH          # 512 query tokens per core
NT = S // 128                      # 16 key tiles of 128
ND = D // 128                      # 8 dim tiles of 128
NPAIR = H // 2                     # 8 head pairs
NTQ = TQ // 128                    # 4 query tiles

F32 = mybir.dt.float32
BF16 = mybir.dt.bfloat16
FP8 = mybir.dt.float8e4
NP_FP8 = ml_dtypes.float8_e4m3

WSCALE = 32.0                      # host-side weight scale
DR = mybir.MatmulPerfMode.DoubleRow


def _build_program():
    """Build the SPMD Bass program (same for all 8 cores)."""
    nc = bacc.Bacc("TRN2", target_bir_lowering=False, debug=False,
                   num_devices=N_CORES)

    # ---- DRAM I/O ----
    # x host-pretransposed fp8: xT[p, d, t] = x[t, 128d+p]
    # xq = f32 (x + bo) rows 0..TQ
    xT_d = nc.dram_tensor("xT", (128, ND, S), FP8, kind="ExternalInput").ap()
    xq_d = nc.dram_tensor("xq", (TQ, D), F32, kind="ExternalInput").ap()
    # weights host-packed fp8, x32: wX[p, otile, dtile, c]
    wq_d = nc.dram_tensor("wq", (128, ND, ND, 128), FP8,
                          kind="ExternalInput").ap()
    wk_d = nc.dram_tensor("wk", (128, ND, ND, 128), FP8,
                          kind="ExternalInput").ap()
    wv_d = nc.dram_tensor("wv", (128, ND, ND, 128), FP8,
                          kind="ExternalInput").ap()
    # wo[p, dtile, o] = 32*Wo[128*dtile+p, o]
    wo_d = nc.dram_tensor("wo", (128, ND, D), FP8, kind="ExternalInput").ap()
    # biases host-packed [p, otile] f32 (bv pre-doubled)
    bq_d = nc.dram_tensor("bq", (128, ND), F32, kind="ExternalInput").ap()
    bk_d = nc.dram_tensor("bk", (128, ND), F32, kind="ExternalInput").ap()
    bv_d = nc.dram_tensor("bv", (128, ND), F32, kind="ExternalInput").ap()
    gamma_d = nc.dram_tensor("gamma", (D,), F32, kind="ExternalInput").ap()
    beta_d = nc.dram_tensor("beta", (D,), F32, kind="ExternalInput").ap()
    y_d = nc.dram_tensor("y", (TQ, D), F32, kind="ExternalOutput").ap()

    def bcast_rows(src_row_ap, nrows):
        # replicate a [1, N] AP across nrows partitions (DMA only)
        return bass.AP(tensor=src_row_ap.tensor, offset=src_row_ap.offset,
                       ap=[[0, nrows]] + [list(d) for d in src_row_ap.ap[-1:]])

    with tile.TileContext(nc) as tc:
        from contextlib import ExitStack
        with ExitStack() as ctx:
            # ---- pools ----
            consts = ctx.enter_context(tc.tile_pool(name="consts", bufs=1))
            bigp = ctx.enter_context(tc.tile_pool(name="big", bufs=1))
            wsl = ctx.enter_context(tc.tile_pool(name="wsl", bufs=2))
            ktp = ctx.enter_context(tc.tile_pool(name="ktp", bufs=2))
            vts = ctx.enter_context(tc.tile_pool(name="vts", bufs=2))
            vaug = ctx.enter_context(tc.tile_pool(name="vaug", bufs=2))
            expp = ctx.enter_context(tc.tile_pool(name="expp", bufs=10))
            smallp = ctx.enter_context(tc.tile_pool(name="small", bufs=2))
            ybufp = ctx.enter_context(tc.tile_pool(name="ybuf", bufs=2))

            # PSUM: acc 2x1 + sc 2x2 + av 2x1 = 8 banks
            # (transposes/out-proj share the acc/sc slots)
            ps_acc = ctx.enter_context(
                tc.tile_pool(name="ps_acc", bufs=2, space="PSUM"))
            ps_sc = ctx.enter_context(
                tc.tile_pool(name="ps_sc", bufs=2, space="PSUM"))
            ps_av = ctx.enter_context(
                tc.tile_pool(name="ps_av", bufs=2, space="PSUM"))

            # =========================================================
            # Phase A: x chunks + constants (weights stream per-slice)
            # =========================================================
            xT = bigp.tile([128, ND, S], FP8, tag="xT", name="xT")
            nc.sync.dma_start(out=xT[:, :, 0:512], in_=xT_d[:, :, 0:512])

            bq_t = consts.tile([128, ND], F32)
            bk_t = consts.tile([128, ND], F32)
            bv_t = consts.tile([128, ND], F32)

            ident = consts.tile([128, 128], FP8, name="ident")
            make_identity(nc, ident)
            eps_t = consts.tile([128, 1], F32)
            nc.vector.memset(eps_t, EPS)
            ones_r = consts.tile([128, 64], BF16, name="ones_r")
            nc.vector.memset(ones_r, 1.0)

            # =========================================================
            # Phase B+C: flat software pipeline over (pair, group) slots.
            # Pair p uses Q-proj slice j=p only, so Q slices are produced
            # one pair ahead, interleaved with K/V chunks into the
            # attention groups. Scores weights are zero-padded to 128
            # contraction rows (kTz) so FWL keeps LDWEIGHTS off the
            # critical path; the pair's full qT is the rhs (other head's
            # rows hit zero weights).
            # =========================================================
            qT = bigp.tile([128, ND, TQ], FP8, tag="qT", name="qT")
            # kTz[p%2 slot][he]: [128, slot, he, ktile4, 512]; he0 rows
            # 64:128 and he1 rows 0:64 stay zero forever
            kTz = bigp.tile([128, 2, 2, NT // 4, 512], FP8, tag="kTz",
                            name="kTz")
            nc.gpsimd.memset(kTz[64:128, :, 0, :, :], 0.0)
            nc.gpsimd.memset(kTz[0:64, :, 1, :, :], 0.0)

            outT = bigp.tile([128, ND // 2, NTQ, 2, 128], FP8, tag="outT",
                             name="outT")

            av_va = {}

            def pair_alloc(p):
                """DMA the pair's weight slices; alloc va."""
                wq_s = wsl.tile([128, ND, 128], FP8, tag="wq", name="wq_s")
                nc.sync.dma_start(out=wq_s, in_=wq_d[:, p, :, :])
                wk_s = wsl.tile([128, ND, 128], FP8, tag="wk", name="wk_s")
                nc.sync.dma_start(out=wk_s, in_=wk_d[:, p, :, :])
                wv_s = wsl.tile([128, ND, 128], FP8, tag="wv", name="wv_s")
                nc.sync.dma_start(out=wv_s, in_=wv_d[:, p, :, :])
                va = vaug.tile([128, 2, NT // 2, 2, 80], FP8, tag="va",
                               name=f"va{p}")
                nc.gpsimd.memset(va[:, :, :, :, 64:80], 0.0)
                nc.gpsimd.memset(va[:, :, :, :, 64:65], 1.0 / 32.0)
                av_va[p] = va
                return [wq_s, wk_s, wv_s, va]

            def kv_item(p, st, item):
                """item 0: Q proj; 1-4: K chunks; 5-8: V chunks."""
                wq_s, wk_s, wv_s, va = st
                sl = p % 2
                if item == 0:
                    pq = ps_acc.tile([128, TQ], F32, tag="acc", name="pq")
                    for m in range(ND // 2):
                        nc.tensor.matmul(
                            pq, wq_s[:, 2 * m:2 * m + 2, :],
                            xT[:, 2 * m:2 * m + 2, 0:TQ],
                            start=(m == 0), stop=(m == ND // 2 - 1),
                            perf_mode=DR)
                    nc.vector.tensor_scalar(
                        out=qT[:, p, :], in0=pq, scalar1=1.0 / WSCALE,
                        scalar2=bq_t[:, p:p + 1],
                        op0=mybir.AluOpType.mult, op1=mybir.AluOpType.add)
                elif item <= 4:
                    n = item - 1
                    pk = ps_acc.tile([128, 512], F32, tag="acc", name="pk")
                    for m in range(ND // 2):
                        nc.tensor.matmul(
                            pk, wk_s[:, 2 * m:2 * m + 2, :],
                            xT[:, 2 * m:2 * m + 2, 512 * n:512 * (n + 1)],
                            start=(m == 0), stop=(m == ND // 2 - 1),
                            perf_mode=DR)
                    kt_tmp = vts.tile([128, 512], FP8, tag="ktt",
                                      name="kt_tmp")
                    nc.vector.tensor_scalar(
                        out=kt_tmp, in0=pk, scalar1=1.0 / WSCALE,
                        scalar2=bk_t[:, p:p + 1],
                        op0=mybir.AluOpType.mult, op1=mybir.AluOpType.add)
                    nc.gpsimd.tensor_copy(out=kTz[0:64, sl, 0, n, :],
                                          in_=kt_tmp[0:64, :])
                    nc.gpsimd.tensor_copy(out=kTz[64:128, sl, 1, n, :],
                                          in_=kt_tmp[64:128, :])
                else:
                    n = item - 5
                    pv = ps_acc.tile([128, 512], F32, tag="acc", name="pv")
                    for m in range(ND // 2):
                        nc.tensor.matmul(
                            pv, wv_s[:, 2 * m:2 * m + 2, :],
                            xT[:, 2 * m:2 * m + 2, 512 * n:512 * (n + 1)],
                            start=(m == 0), stop=(m == ND // 2 - 1),
                            perf_mode=DR)
                    vts_t = vts.tile([128, 512], FP8, tag="vts", name="vts_t")
                    # stored V = 2*v_true: (32v)*(1/16) + 2*bv
                    nc.vector.tensor_scalar(
                        out=vts_t, in0=pv, scalar1=1.0 / 16.0,
                        scalar2=bv_t[:, p:p + 1],
                        op0=mybir.AluOpType.mult, op1=mybir.AluOpType.add)
                    for s in range(4):
                        t = 4 * n + s
                        # fp8 PE transpose requires output element step 2
                        pt = ps_acc.tile([128, 256], FP8, tag="acc",
                                         name="pt")
                        ptv = pt.rearrange("p (a b) -> p a b", b=2)[:, :, 0]
                        nc.tensor.transpose(
                            ptv, vts_t[:, 128 * s:128 * (s + 1)], ident)
                        nc.vector.tensor_copy(
                            out=va[:, 0, t // 2, t % 2, 0:64],
                            in_=ptv[:, 0:64])
                        nc.vector.tensor_copy(
                            out=va[:, 1, t // 2, t % 2, 0:64],
                            in_=ptv[:, 64:128])

            def scores_exp(p, g, he):
                psc = ps_sc.tile([128, 2, TQ], F32, tag="sc", name="psc")
                for s2 in range(2):
                    t = 2 * g + s2
                    lhs = kTz[:, p % 2, he, t // 4,
                              128 * (t % 4):128 * (t % 4 + 1)]
                    nc.tensor.matmul(psc[:, s2, :], lhs, qT[:, p, :],
                                     start=True, stop=True)
                ex = expp.tile([128, 2, TQ], FP8, tag="ex", name=f"ex{he}")
                nc.scalar.activation(
                    out=ex, in_=psc,
                    func=mybir.ActivationFunctionType.Exp, scale=SCALE)
                return ex

            def normalize(p, pav):
                # rrec = 32/denom (pav[64] = denom/32), via ones-row outer
                # product + fast reciprocal; outT = pav * rrec = 64*attn_out
                for he in range(2):
                    dns = smallp.tile([128, TQ], BF16, tag="dns", name="dns")
                    nc.vector.tensor_copy(out=dns[64:65, :],
                                          in_=pav[he][64:65, :])
                    if p == NPAIR - 1:
                        # sc slots are held by the pre-accumulated out-proj
                        # groups at this point; acc pool is idle
                        rb_t = ps_acc.tile([128, TQ], F32, tag="acc",
                                           name="rb")
                        rb = rb_t[0:64, :]
                    else:
                        rb_t = ps_sc.tile([128, 2, TQ], F32, tag="sc",
                                          name="rb")
                        rb = rb_t[0:64, 0, :]
                    nc.tensor.matmul(rb, ones_r[64:65, :], dns[64:65, :],
                                     start=True, stop=True)
                    rrec = smallp.tile([64, TQ], F32, tag="rrec",
                                       name="rrec")
                    nc.vector.reciprocal_approx_fast(out=rrec, in_=rb)
                    dst = outT[0:64, p // 2, :, p % 2, :]
                    if he == 0:
                        nc.vector.tensor_mul(
                            out=dst,
                            in0=pav[he][0:64, :].rearrange(
                                "p (i c) -> p i c", c=128),
                            in1=rrec[0:64, :].rearrange(
                                "p (i c) -> p i c", c=128))
                    else:
                        tmp = smallp.tile([128, TQ], FP8, tag="otmp",
                                          name="tmp")
                        nc.vector.tensor_mul(
                            out=tmp[0:64, :],
                            in0=pav[he][0:64, :], in1=rrec[0:64, :])
                        nc.gpsimd.dma_start(
                            out=outT[64:128, p // 2, :, p % 2, :],
                            in_=tmp[0:64, :].rearrange(
                                "p (i c) -> p i c", c=128))

            NG = NT // 2          # 8 groups per pair
            LAG = 4               # AV trails scores by 4 slots

            # prelude: pair 0's Q proj + first K chunk; the rest of pair
            # 0's items ride its own early groups (sched0)
            st = pair_alloc(0)
            nc.sync.dma_start(out=bq_t, in_=bq_d)
            nc.sync.dma_start(out=bk_t, in_=bk_d)
            nc.sync.dma_start(out=bv_t, in_=bv_d)
            for c in range(1, 4):
                nc.sync.dma_start(out=xT[:, :, 512 * c:512 * (c + 1)],
                                  in_=xT_d[:, :, 512 * c:512 * (c + 1)])
            kv_item(0, st, 0)
            kv_item(0, st, 1)
            st_next = None
            # pair-0 self items per group: K1,K2,K3+V0,V1,V2,V3
            sched0 = [[2], [3], [4, 5], [6], [7], [8], [], []]
            # next-pair items per group: Q+K0 first, front-loaded
            sched = [[0, 1], [2], [3], [4], [5], [6], [7], [8]]

            # tail-phase loads on the gpsimd queue (issued early; the
            # queue drains them before the first normalize shift)
            wo_t = bigp.tile([128, ND, D], FP8, tag="wo", name="wo_t")
            nc.gpsimd.dma_start(out=wo_t, in_=wo_d)
            xq_t = bigp.tile([128, NTQ, D], F32, tag="xq", name="xq_t")
            nc.gpsimd.dma_start(
                out=xq_t, in_=xq_d.rearrange("(i p) d -> p i d", p=128))
            lnc = bigp.tile([128, 2, D], F32, tag="lnc", name="lnc")
            nc.gpsimd.dma_start(out=lnc[:, 0, :],
                                in_=bcast_rows(gamma_d[None], 128))
            nc.gpsimd.dma_start(out=lnc[:, 1, :],
                                in_=bcast_rows(beta_d[None], 128))
            gamma_b, beta_b = lnc[:, 0, :], lnc[:, 1, :]

            exq = {}
            pav_by_pair = {}
            po_pre = []
            for T in range(NPAIR * NG + LAG):
                if T < NPAIR * NG:
                    p, g = divmod(T, NG)
                    if g == 0 and p + 1 < NPAIR:
                        st_next = pair_alloc(p + 1)
                    for he in range(2):
                        exq[(T, he)] = scores_exp(p, g, he)
                    if p == 0:
                        for item in sched0[g]:
                            kv_item(0, st, item)
                    if p + 1 < NPAIR:
                        for item in sched[g]:
                            kv_item(p + 1, st_next, item)
                    if g == NG - 1 and p + 1 < NPAIR:
                        st = st_next
                    if T == NPAIR * NG - 1:
                        for half in range(2):
                            po = ps_sc.tile([128, 2, TQ], F32, tag="sc",
                                            name="po")
                            for m in range(ND // 2 - 1):
                                nc.tensor.matmul(
                                    po[:, 0, :],
                                    outT[:, m, 0, :, :],
                                    wo_t[:, 2 * m:2 * m + 2,
                                         512 * half:512 * (half + 1)],
                                    start=(m == 0), stop=False,
                                    perf_mode=DR)
                            po_pre.append(po)
                Tav = T - LAG
                if Tav >= 0:
                    pa, ga = divmod(Tav, NG)
                    if ga == 0:
                        pav_by_pair[pa] = [
                            ps_av.tile([128, TQ], F32, tag="av",
                                       name=f"pav{he}") for he in range(2)]
                    pav = pav_by_pair[pa]
                    for he in range(2):
                        nc.tensor.matmul(
                            pav[he][0:80, :],
                            av_va[pa][:, he, ga, :, :], exq.pop((Tav, he)),
                            start=(ga == 0), stop=(ga == NG - 1),
                            perf_mode=DR)
                    if ga == NG - 1:
                        normalize(pa, pav_by_pair.pop(pa))

            # =========================================================
            # Phase D: out-proj + residual + LayerNorm
            # (emitted phase-by-phase across the 4 q-tiles so each
            # engine's in-order queue streams instead of ping-ponging)
            # =========================================================
            ysbs, stats_l, mv_l, rstd_l = [], [], [], []
            for i in range(NTQ):
                ysb = ybufp.tile([128, D], F32, tag="ysb", bufs=4,
                                 name=f"ysb{i}")
                ysbs.append(ysb)
                for half in range(2):
                    if i == 0:
                        po = po_pre[half]
                        ms = [ND // 2 - 1]
                    else:
                        po = ps_sc.tile([128, 2, TQ], F32, tag="sc",
                                        name="po")
                        ms = list(range(ND // 2))
                    for m in ms:
                        nc.tensor.matmul(
                            po[:, 0, :],
                            outT[:, m, i, :, :],
                            wo_t[:, 2 * m:2 * m + 2,
                                 512 * half:512 * (half + 1)],
                            start=(m == 0), stop=(m == ND // 2 - 1),
                            perf_mode=DR)
                    # undo 64 (outT) * 32 (Wo) scaling
                    nc.vector.tensor_scalar(
                        out=ysb[:, 512 * half:512 * (half + 1)],
                        in0=po[:, 0, :],
                        scalar1=1.0 / (64.0 * WSCALE), scalar2=None,
                        op0=mybir.AluOpType.mult)
                # residual (x + bo prefolded host-side)
                nc.vector.tensor_add(out=ysb, in0=ysb, in1=xq_t[:, i, :])
            for i in range(NTQ):
                stats = smallp.tile([128, 2, 6], F32, tag="stats", bufs=4)
                mv = smallp.tile([128, 2], F32, tag="mv", bufs=4)
                yv = ysbs[i].rearrange("p (a b) -> p a b", a=2)
                for sg in range(2):
                    nc.vector.bn_stats(out=stats[:, sg, :], in_=yv[:, sg, :])
                nc.vector.bn_aggr(out=mv, in_=stats)
                stats_l.append(stats)
                mv_l.append(mv)
            for i in range(NTQ):
                sd = smallp.tile([128, 1], F32, tag="sd", bufs=4)
                nc.scalar.activation(out=sd, in_=mv_l[i][:, 1:2],
                                     func=mybir.ActivationFunctionType.Sqrt,
                                     bias=eps_t, scale=1.0)
                rstd = smallp.tile([128, 1], F32, tag="rstd", bufs=4)
                nc.vector.reciprocal(out=rstd, in_=sd)
                rstd_l.append(rstd)
            for i in range(NTQ):
                ysb = ysbs[i]
                nc.vector.tensor_scalar(
                    out=ysb, in0=ysb, scalar1=mv_l[i][:, 0:1],
                    scalar2=rstd_l[i],
                    op0=mybir.AluOpType.subtract, op1=mybir.AluOpType.mult)
                nc.vector.tensor_mul(out=ysb, in0=ysb, in1=gamma_b)
                nc.gpsimd.tensor_add(out=ysb, in0=ysb, in1=beta_b)
                nc.sync.dma_start(out=y_d[128 * i:128 * (i + 1), :], in_=ysb)

    nc.compile()
    return nc


_PROGRAM_CACHE = {}


def _get_program():
    if "p" not in _PROGRAM_CACHE:
        _PROGRAM_CACHE["p"] = _build_program()
    return _PROGRAM_CACHE["p"]


def _pack_w(w):
    # [p, otile, dtile, c] = 32*W[128*dtile+p, 128*otile+c], fp8
    w = (np.asarray(w, np.float32) * WSCALE).reshape(ND, 128, ND, 128)
    return np.ascontiguousarray(w.transpose(1, 2, 0, 3)).astype(NP_FP8)


def _pack_wo(w):
    # [p, dtile, o] = 32*Wo[128*dtile+p, o], fp8
    w = (np.asarray(w, np.float32) * WSCALE).reshape(ND, 128, D)
    return np.ascontiguousarray(w.transpose(1, 0, 2)).astype(NP_FP8)


def _pack_b(b, scale=1.0):
    # [p, otile] = scale*b[128*otile+p]
    b = (np.asarray(b, np.float32) * scale).reshape(ND, 128)
    return np.ascontiguousarray(b.transpose(1, 0))


def kernel(x, Wq, bq, Wk, bk, Wv, bv, Wo, bo, gamma, beta, _trace=False):
    x = np.asarray(x, dtype=np.float32)
    nc = _get_program()

    wq_p, wk_p, wv_p = _pack_w(Wq), _pack_w(Wk), _pack_w(Wv)
    wo_p = _pack_wo(Wo)
    bq_p, bk_p = _pack_b(bq), _pack_b(bk)
    bv_p = _pack_b(bv, 2.0)
    bo_f = np.asarray(bo, np.float32)
    in_maps = []
    for c in range(N_CORES):
        b = c // CORES_PER_BATCH
        off = TQ * (c % CORES_PER_BATCH)
        xb = np.concatenate([x[b, off:], x[b, :off]], axis=0)
        xbT = np.ascontiguousarray(
            xb.T.reshape(ND, 128, S).transpose(1, 0, 2)).astype(NP_FP8)
        in_maps.append({
            "xT": xbT,
            "xq": np.ascontiguousarray(xb[0:TQ]) + bo_f,
            "wq": wq_p, "wk": wk_p, "wv": wv_p, "wo": wo_p,
            "bq": bq_p, "bk": bk_p, "bv": bv_p,
            "gamma": np.asarray(gamma, np.float32),
            "beta": np.asarray(beta, np.float32),
        })

    res = bass_utils.run_bass_kernel_spmd(
        nc, in_maps, list(range(N_CORES)), trace=_trace)

    y = np.empty((B, S, D), dtype=np.float32)
    for c in range(N_CORES):
        b = c // CORES_PER_BATCH
        off = TQ * (c % CORES_PER_BATCH)
        y[b, off:off + TQ] = res.results[c]["y"]

    kernel.last_exec_time_ns = res.exec_time_ns
    return y


kernel.last_exec_time_ns = None


# revision 22
# speedup vs baseline: 1.1984x; 1.1984x over previous
"""Trainium2 Bass kernel: MultiHeadAttention + residual + LayerNorm.

Problem shapes (hardcoded):
  x: (2, 2048, 1024) f32, 16 heads x 64 head_dim, scale = 64**-0.5
  y = LayerNorm(x + MHA(x))

Sharding: token-parallel over 8 cores. Core c handles batch b=c//4 and
query tokens [512*(c%4), 512*(c%4+1)) of that batch. Each core receives
its batch's full token sequence ROTATED so that its own 512 query tokens
are rows 0..511 (attention is permutation-invariant over keys, so K/V
token order does not matter). No cross-core collectives needed.

Datapath: fp8e4 (e4m3) matmul operands everywhere, DoubleRow perf mode
(2 contraction k-tiles per instruction; weights AP [K,2,M] must have
subtile step %16==0) for the 1024-contraction projections and the
256-key AV matmuls. Scores are single-rate fp8 (64-dim contraction
cannot DoubleRow). f32 PSUM accumulation; softmax exp on ScalarE from
PSUM with the 1/8 scale folded in; residual + LayerNorm in f32.

Schedule: next pair's K/V projection chunks are interleaved into the
current pair's attention groups so the PE stream never drains while
ScalarE works through the exps; AV lags scores by one group so the
normalize of pair p overlaps the first scores of pair p+1.

Scale management (fp8 dynamic range):
  W{q,k,v,o} stored x32 (entries ~U(+-1/32) -> ~U(+-1))
  q,k casts multiply PSUM by 1/32 (+bias)
  v cast multiplies by 1/16 (+2*bias) -> stored V = 2*v_true
  va "ones" column = 1/32 -> pav[64] = denom/32; fast recip -> 32/denom
  outT = (2*numer)*(32/denom) = 64*attn_out (sigma ~0.9, fp8-friendly)
  out-proj PSUM is 64*32 = 2048x true value; final cast multiplies 1/2048
  bo is folded into the residual input host-side (xq = x + bo)
"""

import sys

sys.path.insert(0, "/opt/trn_rl_repo")

import numpy as np
import ml_dtypes

import concourse.bass as bass
import concourse.bacc as bacc
import concourse.mybir as mybir
import concourse.tile as tile
from concourse import bass_utils
from concourse.masks import make_identity

# ---- problem constants ----
B = 2
S = 2048
D = 1024
H = 16
DH = 64
SCALE = DH ** -0.5
EPS = 1e-5

N_CORES = 8
CORES_PER_BATCH = N_CORES // B
TQ = S // CORES_PER_BATCH          # 512 query tokens per core
NT = S // 128                      # 16 key tiles of 128
ND = D // 128                      # 8 dim tiles of 128
NPAIR = H // 2                     # 8 head pairs
NTQ = TQ // 128                    # 4 query tiles

F32 = mybir.dt.float32
BF16 = mybir.dt.bfloat16
FP8 = mybir.dt.float8e4
NP_FP8 = ml_dtypes.float8_e4m3

WSCALE = 32.0                      # host-side weight scale
DR = mybir.MatmulPerfMode.DoubleRow


def _build_program():
    """Build the SPMD Bass program (same for all 8 cores)."""
    nc = bacc.Bacc("TRN2", target_bir_lowering=False, debug=False,
                   num_devices=N_CORES)

    # ---- DRAM I/O ----
    # x host-pretransposed fp8: xT[p, d, t] = x[t, 128d+p]
    # xq = f32 (x + bo) rows 0..TQ
    xT_d = nc.dram_tensor("xT", (128, ND, S), FP8, kind="ExternalInput").ap()
    xq_d = nc.dram_tensor("xq", (TQ, D), F32, kind="ExternalInput").ap()
    # weights host-packed fp8, x32: wX[p, otile, dtile, c]
    wq_d = nc.dram_tensor("wq", (128, ND, ND, 128), FP8,
                          kind="ExternalInput").ap()
    wk_d = nc.dram_tensor("wk", (128, ND, ND, 128), FP8,
                          kind="ExternalInput").ap()
    wv_d = nc.dram_tensor("wv", (128, ND, ND, 128), FP8,
                          kind="ExternalInput").ap()
    # wo[p, dtile, o] = 32*Wo[128*dtile+p, o]
    wo_d = nc.dram_tensor("wo", (128, ND, D), FP8, kind="ExternalInput").ap()
    # biases host-packed [p, otile] f32 (bv pre-doubled)
    bq_d = nc.dram_tensor("bq", (128, ND), F32, kind="ExternalInput").ap()
    bk_d = nc.dram_tensor("bk", (128, ND), F32, kind="ExternalInput").ap()
    bv_d = nc.dram_tensor("bv", (128, ND), F32, kind="ExternalInput").ap()
    gamma_d = nc.dram_tensor("gamma", (D,), F32, kind="ExternalInput").ap()
    beta_d = nc.dram_tensor("beta", (D,), F32, kind="ExternalInput").ap()
    y_d = nc.dram_tensor("y", (TQ, D), F32, kind="ExternalOutput").ap()

    def bcast_rows(src_row_ap, nrows):
        # replicate a [1, N] AP across nrows partitions (DMA only)
        return bass.AP(tensor=src_row_ap.tensor, offset=src_row_ap.offset,
                       ap=[[0, nrows]] + [list(d) for d in src_row_ap.ap[-1:]])

    with tile.TileContext(nc) as tc:
        from contextlib import ExitStack
        with ExitStack() as ctx:
            # ---- pools ----
            consts = ctx.enter_context(tc.tile_pool(name="consts", bufs=1))
            bigp = ctx.enter_context(tc.tile_pool(name="big", bufs=1))
            wsl = ctx.enter_context(tc.tile_pool(name="wsl", bufs=2))
            ktp = ctx.enter_context(tc.tile_pool(name="ktp", bufs=2))
            vts = ctx.enter_context(tc.tile_pool(name="vts", bufs=2))
            vaug = ctx.enter_context(tc.tile_pool(name="vaug", bufs=2))
            expp = ctx.enter_context(tc.tile_pool(name="expp", bufs=10))
            smallp = ctx.enter_context(tc.tile_pool(name="small", bufs=2))
            ybufp = ctx.enter_context(tc.tile_pool(name="ybuf", bufs=2))

            # PSUM: acc 2x1 + sc 2x2 + av 2x1 = 8 banks
            # (transposes/out-proj share the acc/sc slots)
            ps_acc = ctx.enter_context(
                tc.tile_pool(name="ps_acc", bufs=2, space="PSUM"))
            ps_sc = ctx.enter_context(
                tc.tile_pool(name="ps_sc", bufs=2, space="PSUM"))
            ps_av = ctx.enter_context(
                tc.tile_pool(name="ps_av", bufs=2, space="PSUM"))

            # =========================================================
            # Phase A: x chunks + constants (weights stream per-slice)
            # =========================================================
            xT = bigp.tile([128, ND, S], FP8, tag="xT", name="xT")
            nc.sync.dma_start(out=xT[:, :, 0:512], in_=xT_d[:, :, 0:512])

            bq_t = consts.tile([128, ND], F32)
            bk_t = consts.tile([128, ND], F32)
            bv_t = consts.tile([128, ND], F32)

            ident = consts.tile([128, 128], FP8, name="ident")
            make_identity(nc, ident)
            eps_t = consts.tile([128, 1], F32)
            nc.vector.memset(eps_t, EPS)
            ones_r = consts.tile([128, 64], BF16, name="ones_r")
            nc.vector.memset(ones_r, 1.0)

            # =========================================================
            # Phase B+C: flat software pipeline over (pair, group) slots.
            # Pair p uses Q-proj slice j=p only, so Q slices are produced
            # one pair ahead, interleaved with K/V chunks into the
            # attention groups. Scores weights are zero-padded to 128
            # contraction rows (kTz) so FWL keeps LDWEIGHTS off the
            # critical path; the pair's full qT is the rhs (other head's
            # rows hit zero weights).
            # =========================================================
            qT = bigp.tile([128, ND, TQ], FP8, tag="qT", name="qT")
            # kTz[p%2 slot][he]: [128, slot, he, ktile4, 512]; he0 rows
            # 64:128 and he1 rows 0:64 stay zero forever
            kTz = bigp.tile([128, 2, 2, NT // 4, 512], FP8, tag="kTz",
                            name="kTz")
            nc.gpsimd.memset(kTz[64:128, :, 0, :, :], 0.0)
            nc.gpsimd.memset(kTz[0:64, :, 1, :, :], 0.0)

            outT = bigp.tile([128, ND // 2, NTQ, 2, 128], FP8, tag="outT",
                             name="outT")

            av_va = {}

            def pair_alloc(p):
                """DMA the pair's weight slices; alloc va."""
                wq_s = wsl.tile([128, ND, 128], FP8, tag="wq", name="wq_s")
                nc.sync.dma_start(out=wq_s, in_=wq_d[:, p, :, :])
                wk_s = wsl.tile([128, ND, 128], FP8, tag="wk", name="wk_s")
                nc.sync.dma_start(out=wk_s, in_=wk_d[:, p, :, :])
                wv_s = wsl.tile([128, ND, 128], FP8, tag="wv", name="wv_s")
                nc.sync.dma_start(out=wv_s, in_=wv_d[:, p, :, :])
                va = vaug.tile([128, 2, NT // 2, 2, 80], FP8, tag="va",
                               name=f"va{p}")
                nc.gpsimd.memset(va[:, :, :, :, 64:80], 0.0)
                nc.gpsimd.memset(va[:, :, :, :, 64:65], 1.0 / 32.0)
                av_va[p] = va
                return [wq_s, wk_s, wv_s, va]

            def kv_item(p, st, item):
                """item 0: Q proj; 1-4: K chunks; 5-8: V chunks."""
                wq_s, wk_s, wv_s, va = st
                sl = p % 2
                if item == 0:
                    pq = ps_acc.tile([128, TQ], F32, tag="acc", name="pq")
                    for m in range(ND // 2):
                        nc.tensor.matmul(
                            pq, wq_s[:, 2 * m:2 * m + 2, :],
                            xT[:, 2 * m:2 * m + 2, 0:TQ],
                            start=(m == 0), stop=(m == ND // 2 - 1),
                            perf_mode=DR)
                    nc.vector.tensor_scalar(
                        out=qT[:, p, :], in0=pq, scalar1=1.0 / WSCALE,
                        scalar2=bq_t[:, p:p + 1],
                        op0=mybir.AluOpType.mult, op1=mybir.AluOpType.add)
                elif item <= 4:
                    n = item - 1
                    pk = ps_acc.tile([128, 512], F32, tag="acc", name="pk")
                    for m in range(ND // 2):
                        nc.tensor.matmul(
                            pk, wk_s[:, 2 * m:2 * m + 2, :],
                            xT[:, 2 * m:2 * m + 2, 512 * n:512 * (n + 1)],
                            start=(m == 0), stop=(m == ND // 2 - 1),
                            perf_mode=DR)
                    kt_tmp = vts.tile([128, 512], FP8, tag="ktt",
                                      name="kt_tmp")
                    nc.vector.tensor_scalar(
                        out=kt_tmp, in0=pk, scalar1=1.0 / WSCALE,
                        scalar2=bk_t[:, p:p + 1],
                        op0=mybir.AluOpType.mult, op1=mybir.AluOpType.add)
                    nc.gpsimd.tensor_copy(out=kTz[0:64, sl, 0, n, :],
                                          in_=kt_tmp[0:64, :])
                    nc.gpsimd.tensor_copy(out=kTz[64:128, sl, 1, n, :],
                                          in_=kt_tmp[64:128, :])
                else:
                    n = item - 5
                    pv = ps_acc.tile([128, 512], F32, tag="acc", name="pv")
                    for m in range(ND // 2):
                        nc.tensor.matmul(
                            pv, wv_s[:, 2 * m:2 * m + 2, :],
                            xT[:, 2 * m:2 * m + 2, 512 * n:512 * (n + 1)],
                            start=(m == 0), stop=(m == ND // 2 - 1),
                            perf_mode=DR)
                    vts_t = vts.tile([128, 512], FP8, tag="vts", name="vts_t")
                    # stored V = 2*v_true: (32v)*(1/16) + 2*bv
                    nc.vector.tensor_scalar(
                        out=vts_t, in0=pv, scalar1=1.0 / 16.0,
                        scalar2=bv_t[:, p:p + 1],
                        op0=mybir.AluOpType.mult, op1=mybir.AluOpType.add)
                    for s in range(4):
                        t = 4 * n + s
                        # fp8 PE transpose requires output element step 2
                        pt = ps_acc.tile([128, 256], FP8, tag="acc",
                                         name="pt")
                        ptv = pt.rearrange("p (a b) -> p a b", b=2)[:, :, 0]
                        nc.tensor.transpose(
                            ptv, vts_t[:, 128 * s:128 * (s + 1)], ident)
                        nc.vector.tensor_copy(
                            out=va[:, 0, t // 2, t % 2, 0:64],
                            in_=ptv[:, 0:64])
                        nc.vector.tensor_copy(
                            out=va[:, 1, t // 2, t % 2, 0:64],
                            in_=ptv[:, 64:128])

            def scores_exp(p, g, he):
                psc = ps_sc.tile([128, 2, TQ], F32, tag="sc", name="psc")
                for s2 in range(2):
                    t = 2 * g + s2
                    lhs = kTz[:, p % 2, he, t // 4,
                              128 * (t % 4):128 * (t % 4 + 1)]
                    nc.tensor.matmul(psc[:, s2, :], lhs, qT[:, p, :],
                                     start=True, stop=True)
                ex = expp.tile([128, 2, TQ], FP8, tag="ex", name=f"ex{he}")
                nc.scalar.activation(
                    out=ex, in_=psc,
                    func=mybir.ActivationFunctionType.Exp, scale=SCALE)
                return ex

            def normalize(p, pav):
                # rrec = 32/denom (pav[64] = denom/32), via ones-row outer
                # product + fast reciprocal; outT = pav * rrec = 64*attn_out
                for he in range(2):
                    dns = smallp.tile([128, TQ], BF16, tag="dns", name="dns")
                    nc.vector.tensor_copy(out=dns[64:65, :],
                                          in_=pav[he][64:65, :])
                    if p == NPAIR - 1:
                        # sc slots are held by the pre-accumulated out-proj
                        # groups at this point; acc pool is idle
                        rb_t = ps_acc.tile([128, TQ], F32, tag="acc",
                                           name="rb")
                        rb = rb_t[0:64, :]
                    else:
                        rb_t = ps_sc.tile([128, 2, TQ], F32, tag="sc",
                                          name="rb")
                        rb = rb_t[0:64, 0, :]
                    nc.tensor.matmul(rb, ones_r[64:65, :], dns[64:65, :],
                                     start=True, stop=True)
                    rrec = smallp.tile([64, TQ], F32, tag="rrec",
                                       name="rrec")
                    nc.vector.reciprocal_approx_fast(out=rrec, in_=rb)
                    dst = outT[0:64, p // 2, :, p % 2, :]
                    if he == 0:
                        nc.vector.tensor_mul(
                            out=dst,
                            in0=pav[he][0:64, :].rearrange(
                                "p (i c) -> p i c", c=128),
                            in1=rrec[0:64, :].rearrange(
                                "p (i c) -> p i c", c=128))
                    else:
                        tmp = smallp.tile([128, TQ], FP8, tag="otmp",
                                          name="tmp")
                        nc.vector.tensor_mul(
                            out=tmp[0:64, :],
                            in0=pav[he][0:64, :], in1=rrec[0:64, :])
                        nc.gpsimd.dma_start(
                            out=outT[64:128, p // 2, :, p % 2, :],
                            in_=tmp[0:64, :].rearrange(
                                "p (i c) -> p i c", c=128))

            NG = NT // 2          # 8 groups per pair
            LAG = 4               # AV trails scores by 4 slots

            # prelude: pair 0's Q proj + first K chunk; the rest of pair
            # 0's items ride its own early groups (sched0)
            st = pair_alloc(0)
            nc.sync.dma_start(out=bq_t, in_=bq_d)
            nc.sync.dma_start(out=bk_t, in_=bk_d)
            nc.sync.dma_start(out=bv_t, in_=bv_d)
            for c in range(1, 4):
                nc.sync.dma_start(out=xT[:, :, 512 * c:512 * (c + 1)],
                                  in_=xT_d[:, :, 512 * c:512 * (c + 1)])
            kv_item(0, st, 0)
            kv_item(0, st, 1)
            st_next = None
            # pair-0 self items per group: K1,K2,K3+V0,V1,V2,V3
            sched0 = [[2], [3], [4, 5], [6], [7], [8], [], []]
            # next-pair items per group: Q+K0 first, front-loaded
            sched = [[0, 1], [2], [3], [4], [5], [6], [7], [8]]

            # tail-phase loads on the gpsimd queue (issued early; the
            # queue drains them before the first normalize shift)
            wo_t = bigp.tile([128, ND, D], FP8, tag="wo", name="wo_t")
            nc.gpsimd.dma_start(out=wo_t, in_=wo_d)
            xq_t = bigp.tile([128, NTQ, D], F32, tag="xq", name="xq_t")
            nc.gpsimd.dma_start(
                out=xq_t, in_=xq_d.rearrange("(i p) d -> p i d", p=128))
            lnc = bigp.tile([128, 2, D], F32, tag="lnc", name="lnc")
            nc.gpsimd.dma_start(out=lnc[:, 0, :],
                                in_=bcast_rows(gamma_d[None], 128))
            nc.gpsimd.dma_start(out=lnc[:, 1, :],
                                in_=bcast_rows(beta_d[None], 128))
            gamma_b, beta_b = lnc[:, 0, :], lnc[:, 1, :]

            exq = {}
            pav_by_pair = {}
            po_pre = []
            for T in range(NPAIR * NG + LAG):
                if T < NPAIR * NG:
                    p, g = divmod(T, NG)
                    if g == 0 and p + 1 < NPAIR:
                        st_next = pair_alloc(p + 1)
                    for he in range(2):
                        exq[(T, he)] = scores_exp(p, g, he)
                    if p == 0:
                        for item in sched0[g]:
                            kv_item(0, st, item)
                    if p + 1 < NPAIR:
                        for item in sched[g]:
                            kv_item(p + 1, st_next, item)
                    if g == NG - 1 and p + 1 < NPAIR:
                        st = st_next
                    if T == NPAIR * NG - 1:
                        for half in range(2):
                            po = ps_sc.tile([128, 2, TQ], F32, tag="sc",
                                            name="po")
                            for m in range(ND // 2 - 1):
                                nc.tensor.matmul(
                                    po[:, 0, :],
                                    outT[:, m, 0, :, :],
                                    wo_t[:, 2 * m:2 * m + 2,
                                         512 * half:512 * (half + 1)],
                                    start=(m == 0), stop=False,
                                    perf_mode=DR)
                            po_pre.append(po)
                Tav = T - LAG
                if Tav >= 0:
                    pa, ga = divmod(Tav, NG)
                    if ga == 0:
                        pav_by_pair[pa] = [
                            ps_av.tile([128, TQ], F32, tag="av",
                                       name=f"pav{he}") for he in range(2)]
                    pav = pav_by_pair[pa]
                    for he in range(2):
                        nc.tensor.matmul(
                            pav[he][0:80, :],
                            av_va[pa][:, he, ga, :, :], exq.pop((Tav, he)),
                            start=(ga == 0), stop=(ga == NG - 1),
                            perf_mode=DR)
                    if ga == NG - 1:
                        normalize(pa, pav_by_pair.pop(pa))

            # =========================================================
            # Phase D: out-proj + residual + LayerNorm
            # (emitted phase-by-phase across the 4 q-tiles so each
            # engine's in-order queue streams instead of ping-ponging)
            # =========================================================
            ysbs, stats_l, mv_l, rstd_l = [], [], [], []
            for i in range(NTQ):
                ysb = ybufp.tile([128, D], F32, tag="ysb", bufs=4,
                                 name=f"ysb{i}")
                ysbs.append(ysb)
                for half in range(2):
                    if i == 0:
                        po = po_pre[half]
                        ms = [ND // 2 - 1]
                    else:
                        po = ps_sc.tile([128, 2, TQ], F32, tag="sc",
                                        name="po")
                        ms = list(range(ND // 2))
                    for m in ms:
                        nc.tensor.matmul(
                            po[:, 0, :],
                            outT[:, m, i, :, :],
                            wo_t[:, 2 * m:2 * m + 2,
                                 512 * half:512 * (half + 1)],
                            start=(m == 0), stop=(m == ND // 2 - 1),
                            perf_mode=DR)
                    # undo 64 (outT) * 32 (Wo) scaling
                    nc.vector.tensor_scalar(
                        out=ysb[:, 512 * half:512 * (half + 1)],
                        in0=po[:, 0, :],
                        scalar1=1.0 / (64.0 * WSCALE), scalar2=None,
                        op0=mybir.AluOpType.mult)
                # residual (x + bo prefolded host-side)
                nc.vector.tensor_add(out=ysb, in0=ysb, in1=xq_t[:, i, :])
            for i in range(NTQ):
                stats = smallp.tile([128, 2, 6], F32, tag="stats", bufs=4)
                mv = smallp.tile([128, 2], F32, tag="mv", bufs=4)
                yv = ysbs[i].rearrange("p (a b) -> p a b", a=2)
                for sg in range(2):
                    nc.vector.bn_stats(out=stats[:, sg, :], in_=yv[:, sg, :])
                nc.vector.bn_aggr(out=mv, in_=stats)
                stats_l.append(stats)
                mv_l.append(mv)
            for i in range(NTQ):
                sd = smallp.tile([128, 1], F32, tag="sd", bufs=4)
                nc.scalar.activation(out=sd, in_=mv_l[i][:, 1:2],
                                     func=mybir.ActivationFunctionType.Sqrt,
                                     bias=eps_t, scale=1.0)
                rstd = smallp.tile([128, 1], F32, tag="rstd", bufs=4)
                nc.vector.reciprocal(out=rstd, in_=sd)
                rstd_l.append(rstd)
            for i in range(NTQ):
                ysb = ysbs[i]
                nc.vector.tensor_scalar(
                    out=ysb, in0=ysb, scalar1=mv_l[i][:, 0:1],
                    scalar2=rstd_l[i],
                    op0=mybir.AluOpType.subtract, op1=mybir.AluOpType.mult)
                nc.vector.tensor_mul(out=ysb, in0=ysb, in1=gamma_b)
                nc.gpsimd.tensor_add(out=ysb, in0=ysb, in1=beta_b)
                nc.sync.dma_start(out=y_d[128 * i:128 * (i + 1), :], in_=ysb)

    nc.compile()
    return nc


_PROGRAM_CACHE = {}


def _get_program():
    if "p" not in _PROGRAM_CACHE:
        _PROGRAM_CACHE["p"] = _build_program()
    return _PROGRAM_CACHE["p"]


def _pack_w(w):
    # [p, otile, dtile, c] = 32*W[128*dtile+p, 128*otile+c], fp8
    w = (np.asarray(w, np.float32) * WSCALE).reshape(ND, 128, ND, 128)
    return np.ascontiguousarray(w.transpose(1, 2, 0, 3)).astype(NP_FP8)


def _pack_wo(w):
    # [p, dtile, o] = 32*Wo[128*dtile+p, o], fp8
    w = (np.asarray(w, np.float32) * WSCALE).reshape(ND, 128, D)
    return np.ascontiguousarray(w.transpose(1, 0, 2)).astype(NP_FP8)


def _pack_b(b, scale=1.0):
    # [p, otile] = scale*b[128*otile+p]
    b = (np.asarray(b, np.float32) * scale).reshape(ND, 128)
    return np.ascontiguousarray(b.transpose(1, 0))


def kernel(x, Wq, bq, Wk, bk, Wv, bv, Wo, bo, gamma, beta, _trace=False):
    x = np.asarray(x, dtype=np.float32)
    nc = _get_program()

    wq_p, wk_p, wv_p = _pack_w(Wq), _pack_w(Wk), _pack_w(Wv)
    wo_p = _pack_wo(Wo)
    bq_p, bk_p = _pack_b(bq), _pack_b(bk)
    bv_p = _pack_b(bv, 2.0)
    bo_f = np.asarray(bo, np.float32)
    in_maps = []
    for c in range(N_CORES):
        b = c // CORES_PER_BATCH
        off = TQ * (c % CORES_PER_BATCH)
        xb = np.concatenate([x[b, off:], x[b, :off]], axis=0)
        xbT = np.ascontiguousarray(
            xb.T.reshape(ND, 128, S).transpose(1, 0, 2)).astype(NP_FP8)
        in_maps.append({
            "xT": xbT,
            "xq": np.ascontiguousarray(xb[0:TQ]) + bo_f,
            "wq": wq_p, "wk": wk_p, "wv": wv_p, "wo": wo_p,
            "bq": bq_p, "bk": bk_p, "bv": bv_p,
            "gamma": np.asarray(gamma, np.float32),
            "beta": np.asarray(beta, np.float32),
        })

    res = bass_utils.run_bass_kernel_spmd(
        nc, in_maps, list(range(N_CORES)), trace=_trace)

    y = np.empty((B, S, D), dtype=np.float32)
    for c in range(N_CORES):
        b = c // CORES_PER_BATCH
        off = TQ * (c % CORES_PER_BATCH)
        y[b, off:off + TQ] = res.results[c]["y"]

    kernel.last_exec_time_ns = res.exec_time_ns
    return y


kernel.last_exec_time_ns = None
